# revision 26
# baseline (speedup 1.0000x reference)
"""EpisodicMemory kernel for Trainium2, 8-core data-parallel, bf16 compute.

Reference computation (per batch b, d=32, m=64 memory slots, 2 hops):
    M = vs[b]
    for hop:
        Rh[m,:] = R[b,hop,m] @ h[b,hop,m]                  # batched matvec
        z = [Rh*v, Rh*M, |Rh-v|, |Rh-M|]                   # [m, 4d]
        Z = tanh(z @ W1.T + b1) @ W2.T (+ b2: dropped — softmax-invariant)
        g = softmax(Z over m); o = sum_m ts[b,hop,m] * g[m]
        M = GRUCell(o, M)
    out[b] = M

Sharding: pure data parallel over batch; 128 batches per core.

bf16 strategy (tolerance 2e-2; emulated end-to-end rel err ~2.7e-3):
  - Rs/hs/ts host-converted to bf16: halves the dominant HBM traffic
    (Rs 64 MiB -> 32 MiB per core) and enables DVE 4x perf mode.
  - einsum mul+reduce in bf16 on DVE (odd groups' mul on GPSIMD),
    features bf16, MLP matmuls bf16 (PSUM fp32), softmax stats fp32,
    GRU fp32.

Per-core layout: b = blk*8 + bp*4 + g; partition p = bp*64 + m.
  - R tiles [128 part=(bp,m), free=(blk2,g4,d32,e32)]: 2 blocks per DMA,
    rotated across SP/Act HWDGE queues.
  - z written per block as one contiguous 1 KB DMA; gathered per hop as
    one clean [128, 64] load (p-order (bp,m) makes both sides packed).
  - v_rep/M_rep [128, (blk g d)] bf16; v_rep host-prebuilt, M_rep via
    3-DMA DRAM bounce after hop 0.
  - weights host-pretransposed (W1T/W2T bf16, WihT/WhhT fp32).
"""

import numpy as np
import ml_dtypes

import concourse.bacc as bacc
import concourse.bass as bass
import concourse.mybir as mybir
import concourse.tile as tile
from concourse.masks import make_identity

F32 = mybir.dt.float32
BF16 = mybir.dt.bfloat16
AF = mybir.ActivationFunctionType
ALU = mybir.AluOpType
AX = mybir.AxisListType

B, N_HOP, N_MEM, DIM = 1024, 2, 64, 32
N_CORES = 8
BC = B // N_CORES            # 128 batches per core
NBLK = 16                    # blocks of 8 batches
NG = 4                       # g per bp
D4 = 4 * DIM                 # 128 MLP input features
RB = 4                       # blocks per R DMA
NGRP = NBLK // RB            # 4 groups per hop
ROWS = 512                   # zt columns per block (4 g-chunks x 128)

R_QUEUES = ("sync", "scalar")    # rotate r-tile DMA trigger queues
GPSIMD_MUL = True                # odd groups' R*h mul on Pool engine
ABS_ON_DVE = True                # |x| via tensor_scalar abs_max on DVE


def build_nc(n_iter: int = 1) -> bass.Bass:
    nc = bacc.Bacc("TRN2")

    Rs_d = nc.dram_tensor(
        "Rs", [N_HOP, NBLK, 2, N_MEM, NG, DIM, DIM], BF16, kind="ExternalInput"
    )
    hs_d = nc.dram_tensor(
        "hs", [N_HOP, 2, N_MEM, NBLK, NG, DIM], BF16, kind="ExternalInput"
    )
    ts_d = nc.dram_tensor("ts", [BC, N_HOP, DIM, N_MEM], BF16, kind="ExternalInput")
    vrep_d = nc.dram_tensor(
        "v_rep", [128, NBLK * NG * DIM], BF16, kind="ExternalInput"
    )
    vs_d = nc.dram_tensor("vs", [BC, DIM], F32, kind="ExternalInput")
    W1T_d = nc.dram_tensor("W1T", [D4, DIM], BF16, kind="ExternalInput")
    W2T_d = nc.dram_tensor("W2T", [DIM, 1], BF16, kind="ExternalInput")
    b1_d = nc.dram_tensor("b1", [DIM], F32, kind="ExternalInput")
    WihT_d = nc.dram_tensor("WihT", [N_HOP, DIM, 3 * DIM], F32, kind="ExternalInput")
    WhhT_d = nc.dram_tensor("WhhT", [N_HOP, DIM, 3 * DIM], F32, kind="ExternalInput")
    bih_d = nc.dram_tensor("b_ih", [N_HOP, 3 * DIM], F32, kind="ExternalInput")
    bhh_d = nc.dram_tensor("b_hh", [N_HOP, 3 * DIM], F32, kind="ExternalInput")
    out_d = nc.dram_tensor("out", [BC, DIM], F32, kind="ExternalOutput")
    m_scr = nc.dram_tensor("m_scratch", [BC, DIM], BF16)
    m_scr2 = nc.dram_tensor("m_scratch2", [2, NBLK, NG, DIM], BF16)
    m_scr3 = nc.dram_tensor("m_scratch3", [128, NBLK * NG * DIM], BF16)
    z_scr = nc.dram_tensor("z_scratch", [NBLK, 2, NG, N_MEM], BF16)

    import contextlib

    with tile.TileContext(nc) as tc:
        with (
            (tc.For_i(0, n_iter, 1) if n_iter > 1 else contextlib.nullcontext()),
            tc.tile_pool(name="consts", bufs=1) as consts,
            tc.tile_pool(name="hop_io", bufs=2) as hop_io,
            tc.tile_pool(name="rpool", bufs=3) as rpool,
            tc.tile_pool(name="fpool", bufs=3) as fpool,
            tc.tile_pool(name="zpool", bufs=3) as zpool,
            tc.tile_pool(name="apool", bufs=3) as apool,
            tc.tile_pool(name="small", bufs=2) as small,
            tc.tile_pool(name="mstate", bufs=2) as mstate,
            tc.tile_pool(name="pp_z", bufs=2, space="PSUM") as pp_z,
            tc.tile_pool(name="pp_1", bufs=2, space="PSUM") as pp_1,
            tc.tile_pool(name="pp_2", bufs=2, space="PSUM") as pp_2,
            tc.tile_pool(name="pp_g", bufs=2, space="PSUM") as pp_g,
        ):
            ident = consts.tile([128, 128], F32)
            make_identity(nc, ident)
            ident_bf = consts.tile([128, 128], BF16)
            make_identity(nc, ident_bf)

            # preload h/t for both hops (Act queue) so the hop-1 einsum never
            # stalls on them behind hop-0's z writes
            h_hops, t_hops = [], []
            for hop in range(N_HOP):
                h_hop = hop_io.tile([128, NBLK * NG * DIM], BF16, tag="h_hop")
                nc.scalar.dma_start(
                    out=h_hop,
                    in_=hs_d[hop].rearrange("bp m blk g e -> (bp m) (blk g e)"),
                )
                h_hops.append(h_hop)
                # t d-major [b, (d m)] so the m-reduce is innermost-packed
                t_hop = hop_io.tile([BC, DIM * N_MEM], BF16, tag="t_hop")
                nc.scalar.dma_start(
                    out=t_hop, in_=ts_d[:, hop].rearrange("b d m -> b (d m)")
                )
                t_hops.append(t_hop)

            # ---- weights (host-pretransposed, direct loads) ----
            W1T = consts.tile([D4, DIM], BF16)
            nc.scalar.dma_start(out=W1T, in_=W1T_d[:, :])
            W2T = consts.tile([DIM, 1], BF16)
            nc.scalar.dma_start(out=W2T, in_=W2T_d[:, :])
            b1T = consts.tile([DIM, 1], F32)
            nc.scalar.dma_start(out=b1T, in_=b1_d[:].unsqueeze(1))

            WihT, WhhT, bsum_rz, bihn_t, bhhn_t = [], [], [], [], []
            for hop in range(N_HOP):
                wT = consts.tile([DIM, 3 * DIM], F32, tag=f"wihT{hop}")
                nc.scalar.dma_start(out=wT, in_=WihT_d[hop])
                WihT.append(wT)
                wT2 = consts.tile([DIM, 3 * DIM], F32, tag=f"whhT{hop}")
                nc.scalar.dma_start(out=wT2, in_=WhhT_d[hop])
                WhhT.append(wT2)

                gate_b = []
                for gd, gname in ((bih_d, "ih"), (bhh_d, "hh")):
                    for gate in range(3):
                        bt = consts.tile([DIM, 1], F32, tag=f"b{gname}{hop}{gate}")
                        nc.scalar.dma_start(
                            out=bt,
                            in_=gd[hop, gate * DIM : (gate + 1) * DIM].unsqueeze(1),
                        )
                        gate_b.append(bt)
                b_r = consts.tile([DIM, 1], F32, tag=f"b_r{hop}")
                nc.vector.tensor_add(b_r, gate_b[0], gate_b[3])
                b_z = consts.tile([DIM, 1], F32, tag=f"b_z{hop}")
                nc.vector.tensor_add(b_z, gate_b[1], gate_b[4])
                bsum_rz.append((b_r, b_z))
                bihn_t.append(gate_b[2])
                bhhn_t.append(gate_b[5])

            # ---- initial M state ----
            vs_row = consts.tile([BC, DIM], F32)
            nc.scalar.dma_start(out=vs_row, in_=vs_d[:, :])
            vst_ps = pp_g.tile([DIM, BC], F32, tag="gpsum")
            nc.tensor.transpose(vst_ps, vs_row, ident)
            vsT = consts.tile([DIM, BC], F32)
            nc.scalar.copy(out=vsT, in_=vst_ps)
            MT = vsT  # current M^T [d, b]

            v_rep = consts.tile([128, NBLK * NG * DIM], BF16)
            nc.scalar.dma_start(out=v_rep, in_=vrep_d[:, :])
            M_rep = v_rep  # hop 0: M == vs

            for hop in range(N_HOP):
                h_hop = h_hops[hop]
                t_hop = t_hops[hop]
                deferred_z = []

                for grp in range(NGRP):
                    blk0 = grp * RB
                    r_tile = rpool.tile([128, RB * NG * DIM * DIM], BF16, tag="R")
                    # SP carries ONLY the R stream: its in-order SEQ must
                    # never wait on downstream compute
                    nc.sync.dma_start(
                        out=r_tile.rearrange("p (blk f) -> p blk f", blk=RB),
                        in_=Rs_d[hop, blk0 : blk0 + RB].rearrange(
                            "blk bp m g d e -> (bp m) blk (g d e)"
                        ),
                    )
                    # P = R * h (in-place), h broadcast over d
                    r5 = r_tile.rearrange(
                        "p (blk g d e) -> p blk g d e", blk=RB, g=NG, d=DIM
                    )
                    h_v = (
                        h_hop[:, blk0 * NG * DIM : (blk0 + RB) * NG * DIM]
                        .rearrange("p (blk g e) -> p blk g e", blk=RB, g=NG)
                        .unsqueeze(3)
                        .broadcast_to((128, RB, NG, DIM, DIM))
                    )
                    # Pool owns every mul, DVE owns every tree+features: each
                    # engine's in-order stream then pipelines across groups
                    # (mul N+1 on Pool overlaps tree N on DVE).
                    nc.gpsimd.tensor_tensor(r5, r5, h_v, op=ALU.mult)
                    # Rh[(bp,m), (blk,g,d)] = sum_e P via in-place halving
                    # tree: TensorTensor adds run at 2 elem/cycle on DVE vs
                    # TensorReduce's 1.
                    rh = fpool.tile([128, RB * NG * DIM], BF16, tag="rh")
                    v3 = r_tile.rearrange("p (gd e) -> p gd e", e=DIM)
                    for w in (16, 8, 4, 2):
                        nc.vector.tensor_add(
                            v3[:, :, 0:w], v3[:, :, 0:w], v3[:, :, w : 2 * w]
                        )
                    nc.vector.tensor_add(rh, v3[:, :, 0], v3[:, :, 1])
                    # features F [(bp,m), (blk, g, f, d)]
                    f_grp = fpool.tile([128, RB * NG * 4 * DIM], BF16, tag="F")
                    f5 = f_grp.rearrange(
                        "p (blk g f d) -> p blk g f d", blk=RB, g=NG, f=4
                    )
                    rh4 = rh.rearrange("p (blk g d) -> p blk g d", blk=RB, g=NG)
                    vr4 = v_rep[
                        :, blk0 * NG * DIM : (blk0 + RB) * NG * DIM
                    ].rearrange("p (blk g d) -> p blk g d", blk=RB, g=NG)
                    mr4 = M_rep[
                        :, blk0 * NG * DIM : (blk0 + RB) * NG * DIM
                    ].rearrange("p (blk g d) -> p blk g d", blk=RB, g=NG)
                    feng = nc.vector
                    feng.tensor_tensor(f5[:, :, :, 0, :], rh4, vr4, op=ALU.mult)
                    feng.tensor_tensor(f5[:, :, :, 1, :], rh4, mr4, op=ALU.mult)
                    feng.tensor_tensor(f5[:, :, :, 2, :], rh4, vr4, op=ALU.subtract)
                    feng.tensor_tensor(f5[:, :, :, 3, :], rh4, mr4, op=ALU.subtract)
                    nc.scalar.activation(
                        f5[:, :, :, 2, :], f5[:, :, :, 2, :], AF.Abs
                    )
                    nc.scalar.activation(
                        f5[:, :, :, 3, :], f5[:, :, :, 3, :], AF.Abs
                    )

                    for pair in range(RB // 2):
                        # transpose 2 blocks into one PSUM bank ([128, 1024]
                        # bf16 = 2 KB/partition) -> single Act copy-out
                        zt_ps = pp_z.tile([D4, 2 * ROWS], BF16, tag="zt")
                        for j in range(2):
                            blk = pair * 2 + j
                            for g in range(NG):
                                c0 = (blk * NG + g) * 128
                                nc.tensor.transpose(
                                    zt_ps[
                                        :, j * ROWS + g * 128 : j * ROWS + (g + 1) * 128
                                    ],
                                    f_grp[:, c0 : c0 + 128],
                                    ident_bf,
                                )
                        zt_sb = zpool.tile([D4, 2 * ROWS], BF16, tag="zt_sb")
                        nc.scalar.copy(out=zt_sb, in_=zt_ps)

                        for j in range(2):
                            blk = blk0 + pair * 2 + j
                            ps1 = pp_1.tile([DIM, ROWS], F32, tag="ps1")
                            nc.tensor.matmul(
                                ps1,
                                lhsT=W1T,
                                rhs=zt_sb[:, j * ROWS : (j + 1) * ROWS],
                                start=True,
                                stop=True,
                            )
                            a1 = apool.tile([DIM, ROWS], BF16, tag="a1")
                            nc.scalar.activation(a1, ps1, AF.Tanh, bias=b1T)
                            ps2 = pp_2.tile([1, ROWS], F32, tag="ps2")
                            nc.tensor.matmul(
                                ps2, lhsT=W2T, rhs=a1, start=True, stop=True
                            )
                            z_sb = zpool.tile(
                                [1, ROWS], BF16, tag=f"z_sb{hop}",
                                bufs=(NBLK if hop == N_HOP - 1 else 3),
                            )
                            nc.scalar.copy(out=z_sb, in_=ps2)
                            # z_sb free order (g, bp, m); store as (bp, g, m)
                            # so the gather's partition dim is adjacent.
                            # hop 0: write now on Act. last hop: defer to SP
                            # (after its R stream) so Act is not the serializer.
                            if hop < N_HOP - 1:
                                nc.scalar.dma_start(
                                    out=z_scr[blk]
                                    .unsqueeze(0)
                                    .rearrange("o bp g m -> o g bp m"),
                                    in_=z_sb.rearrange(
                                        "o (g bp m) -> o g bp m", g=NG, bp=2
                                    ),
                                )
                            else:
                                deferred_z.append((blk, z_sb))

                if hop == N_HOP - 1:
                    for blk, z_sb in deferred_z:
                        nc.sync.dma_start(
                            out=z_scr[blk]
                            .unsqueeze(0)
                            .rearrange("o bp g m -> o g bp m"),
                            in_=z_sb.rearrange(
                                "o (g bp m) -> o g bp m", g=NG, bp=2
                            ),
                        )

                # gather Z rows: flat (blk,bp,g) == natural b, m packed
                Z_row = small.tile([BC, N_MEM], BF16, tag="Z_row")
                zq = nc.sync if hop == N_HOP - 1 else nc.scalar
                zq.dma_start(
                    out=Z_row,
                    in_=z_scr.rearrange("blk bp g m -> (blk bp g) m"),
                )

                # softmax over m, batched over all 128 b
                nmx = small.tile([BC, 1], F32, tag="nmx")
                nc.vector.tensor_reduce(
                    out=nmx, in_=Z_row, axis=AX.X, op=ALU.max, negate=True
                )
                e_row = small.tile([BC, N_MEM], F32, tag="e_row")
                nc.scalar.activation(e_row, Z_row, AF.Exp, bias=nmx)
                ssum = small.tile([BC, 1], F32, tag="ssum")
                nc.vector.tensor_reduce(out=ssum, in_=e_row, axis=AX.X, op=ALU.add)
                rsum = small.tile([BC, 1], F32, tag="rsum")
                nc.vector.reciprocal(rsum, ssum)
                g_bf = small.tile([BC, N_MEM], BF16, tag="g_bf")
                nc.vector.tensor_scalar_mul(g_bf, e_row, rsum)

                # o[b,d] = sum_m t[b,d,m] * g[b,m]  (in-place mul into t_hop)
                t3 = t_hop.rearrange("b (d m) -> b d m", d=DIM)
                g3 = g_bf.unsqueeze(1).broadcast_to((BC, DIM, N_MEM))
                nc.vector.tensor_mul(t3, t3, g3)
                for w in (32, 16, 8, 4, 2):
                    nc.vector.tensor_add(
                        t3[:, :, 0:w], t3[:, :, 0:w], t3[:, :, w : 2 * w]
                    )
                o_row = small.tile([BC, DIM], F32, tag="o_row")
                nc.vector.tensor_add(o_row, t3[:, :, 0], t3[:, :, 1])

                # GRU (transposed layout [*, b], fp32)
                ot_ps = pp_g.tile([DIM, BC], F32, tag="gpsum")
                nc.tensor.transpose(ot_ps, o_row, ident)
                oT = small.tile([DIM, BC], F32, tag="oT")
                nc.scalar.copy(out=oT, in_=ot_ps)

                def gate_pair(g):
                    gi = pp_g.tile([DIM, BC], F32, tag="gpsum")
                    nc.tensor.matmul(
                        gi,
                        lhsT=WihT[hop][:, g * DIM : (g + 1) * DIM],
                        rhs=oT,
                        start=True,
                        stop=True,
                    )
                    gh = pp_g.tile([DIM, BC], F32, tag="gpsum")
                    nc.tensor.matmul(
                        gh,
                        lhsT=WhhT[hop][:, g * DIM : (g + 1) * DIM],
                        rhs=MT,
                        start=True,
                        stop=True,
                    )
                    return gi, gh

                rz_t = []
                for g in range(2):
                    gi, gh = gate_pair(g)
                    gb = small.tile([DIM, BC], F32, tag=f"g{g}b")
                    nc.vector.tensor_scalar(
                        out=gb, in0=gi, scalar1=bsum_rz[hop][g], scalar2=None,
                        op0=ALU.add,
                    )
                    nc.vector.tensor_add(gb, gb, gh)
                    gt = small.tile([DIM, BC], F32, tag=f"gate{g}")
                    # sigmoid via tanh keeps every Act func in the
                    # exp_and_others table: no LoadActFuncSet swaps
                    nc.scalar.activation(gt, gb, AF.Tanh, scale=0.5)
                    nc.vector.tensor_scalar(
                        out=gt, in0=gt, scalar1=0.5, scalar2=0.5,
                        op0=ALU.mult, op1=ALU.add,
                    )
                    rz_t.append(gt)
                r_t, z_t = rz_t

                gi_n, gh_n = gate_pair(2)
                ghn = small.tile([DIM, BC], F32, tag="ghn")
                nc.vector.tensor_scalar(
                    out=ghn, in0=gh_n, scalar1=bhhn_t[hop], scalar2=None,
                    op0=ALU.add,
                )
                gin = small.tile([DIM, BC], F32, tag="gin")
                nc.vector.tensor_scalar(
                    out=gin, in0=gi_n, scalar1=bihn_t[hop], scalar2=None,
                    op0=ALU.add,
                )
                n1 = small.tile([DIM, BC], F32, tag="n1")
                nc.vector.tensor_mul(n1, r_t, ghn)
                nc.vector.tensor_add(n1, n1, gin)
                n_t = small.tile([DIM, BC], F32, tag="n_t")
                nc.scalar.activation(n_t, n1, AF.Tanh)

                # M' = n + z * (M - n)
                MT_new = mstate.tile([DIM, BC], F32, tag="MT")
                nc.vector.tensor_sub(MT_new, MT, n_t)
                nc.vector.tensor_mul(MT_new, MT_new, z_t)
                nc.vector.tensor_add(MT_new, MT_new, n_t)
                MT = MT_new

                # M_row for output / M_rep rebuild
                mrow_ps = pp_g.tile([BC, DIM], F32, tag="gpsum")
                nc.tensor.transpose(mrow_ps, MT, ident[:DIM, :DIM])
                M_row = mstate.tile([BC, DIM], F32, tag="M_row")
                nc.scalar.copy(out=M_row, in_=mrow_ps)

                if hop < N_HOP - 1:
                    # rebuild M_rep (bf16) via DRAM bounce
                    Mb = mstate.tile([BC, DIM], BF16, tag="Mb")
                    nc.scalar.copy(out=Mb, in_=M_row)
                    nc.scalar.dma_start(out=m_scr[:, :], in_=Mb)
                    # DRAM->DRAM permute: natural b -> [bp, blk, g, d]
                    nc.scalar.dma_start(
                        out=m_scr2[:, :, :, :],
                        in_=m_scr.rearrange(
                            "(blk bp g) d -> bp blk g d", bp=2, g=NG
                        ),
                    )
                    # replicate rows to (bp, m) partitions in DRAM
                    # partition_broadcast prepends the broadcast dim: in is
                    # [m(bcast), bp, f]; write rows r = bp*64+m accordingly
                    nc.scalar.dma_start(
                        out=m_scr3.rearrange("(bp m) f -> m bp f", bp=2),
                        in_=m_scr2.rearrange(
                            "bp blk g d -> bp (blk g d)"
                        ).partition_broadcast(N_MEM),
                    )
                    M_rep_new = mstate.tile(
                        [128, NBLK * NG * DIM], BF16, tag="M_rep", bufs=1
                    )
                    nc.scalar.dma_start(out=M_rep_new, in_=m_scr3[:, :])
                    M_rep = M_rep_new
                else:
                    nc.sync.dma_start(out=out_d[:, :], in_=M_row)

    nc.compile()
    return nc


_NC_CACHE = None


def _get_nc():
    global _NC_CACHE
    if _NC_CACHE is None:
        _NC_CACHE = build_nc()
    return _NC_CACHE


BF_NP = ml_dtypes.bfloat16


def make_in_maps(hs, Rs, ts, vs, W1, b1, W2, W_ih, W_hh, b_ih, b_hh):
    hs = np.asarray(hs)
    Rs = np.asarray(Rs)
    ts = np.asarray(ts)
    vs = np.asarray(vs)
    W1T = np.ascontiguousarray(np.asarray(W1).T).astype(BF_NP)
    W2T = np.ascontiguousarray(np.asarray(W2).T).astype(BF_NP)
    b1 = np.ascontiguousarray(np.asarray(b1))
    WihT = np.ascontiguousarray(np.asarray(W_ih).transpose(0, 2, 1))
    WhhT = np.ascontiguousarray(np.asarray(W_hh).transpose(0, 2, 1))
    b_ih = np.ascontiguousarray(np.asarray(b_ih))
    b_hh = np.ascontiguousarray(np.asarray(b_hh))

    in_maps = []
    for c in range(N_CORES):
        sl = slice(c * BC, (c + 1) * BC)
        # Rs [BC, hop, m, d, e] -> [hop, blk, bp, m, g, d, e]
        Rp = (
            Rs[sl]
            .reshape(NBLK, 2, NG, N_HOP, N_MEM, DIM, DIM)
            .transpose(3, 0, 1, 4, 2, 5, 6)
            .astype(BF_NP)
        )
        # hs [BC, hop, m, e] -> [hop, bp, m, blk, g, e]
        hp = (
            hs[sl]
            .reshape(NBLK, 2, NG, N_HOP, N_MEM, DIM)
            .transpose(3, 1, 4, 0, 2, 5)
            .astype(BF_NP)
        )
        # ts [BC, hop, m, d] -> [BC, hop, d, m]
        tp = ts[sl].transpose(0, 1, 3, 2).astype(BF_NP)
        vsc = np.ascontiguousarray(vs[sl])
        # v_rep [128=(bp,m), (blk, g, d)]
        vv = vsc.reshape(NBLK, 2, NG, DIM).transpose(1, 0, 2, 3)  # [bp, blk, g, d]
        v_rep = (
            np.broadcast_to(vv[:, None], (2, N_MEM, NBLK, NG, DIM))
            .reshape(128, NBLK * NG * DIM)
            .astype(BF_NP)
        )
        in_maps.append(
            {
                "Rs": np.ascontiguousarray(Rp),
                "hs": np.ascontiguousarray(hp),
                "ts": np.ascontiguousarray(tp),
                "v_rep": np.ascontiguousarray(v_rep),
                "vs": vsc,
                "W1T": W1T,
                "W2T": W2T,
                "b1": b1,
                "WihT": WihT,
                "WhhT": WhhT,
                "b_ih": b_ih,
                "b_hh": b_hh,
            }
        )
    return in_maps


def kernel(hs, Rs, ts, vs, W1, b1, W2, b2, W_ih, W_hh, b_ih, b_hh):
    from concourse.bass_utils import run_bass_kernel_spmd

    nc = _get_nc()
    in_maps = make_in_maps(hs, Rs, ts, vs, W1, b1, W2, W_ih, W_hh, b_ih, b_hh)
    res = run_bass_kernel_spmd(nc, in_maps, list(range(N_CORES)))
    return np.concatenate([r["out"] for r in res.results], axis=0)


# revision 27
# speedup vs baseline: 1.3189x; 1.3189x over previous
"""EpisodicMemory kernel for Trainium2, 8-core data-parallel, bf16 compute.

Reference computation (per batch b, d=32, m=64 memory slots, 2 hops):
    M = vs[b]
    for hop:
        Rh[m,:] = R[b,hop,m] @ h[b,hop,m]                  # batched matvec
        z = [Rh*v, Rh*M, |Rh-v|, |Rh-M|]                   # [m, 4d]
        Z = tanh(z @ W1.T + b1) @ W2.T (+ b2: dropped — softmax-invariant)
        g = softmax(Z over m); o = sum_m ts[b,hop,m] * g[m]
        M = GRUCell(o, M)
    out[b] = M

Sharding: pure data parallel over batch; 128 batches per core.

bf16 strategy (tolerance 2e-2; emulated end-to-end rel err ~2.7e-3):
  - Rs/hs/ts host-converted to bf16: halves the dominant HBM traffic
    (Rs 64 MiB -> 32 MiB per core) and enables DVE 4x perf mode.
  - einsum mul+reduce in bf16 on DVE (odd groups' mul on GPSIMD),
    features bf16, MLP matmuls bf16 (PSUM fp32), softmax stats fp32,
    GRU fp32.

Per-core layout: b = blk*8 + bp*4 + g; partition p = bp*64 + m.
  - R tiles [128 part=(bp,m), free=(blk2,g4,d32,e32)]: 2 blocks per DMA,
    rotated across SP/Act HWDGE queues.
  - z written per block as one contiguous 1 KB DMA; gathered per hop as
    one clean [128, 64] load (p-order (bp,m) makes both sides packed).
  - v_rep/M_rep [128, (blk g d)] bf16; v_rep host-prebuilt, M_rep via
    3-DMA DRAM bounce after hop 0.
  - weights host-pretransposed (W1T/W2T bf16, WihT/WhhT fp32).
"""

import numpy as np
import ml_dtypes

import concourse.bacc as bacc
import concourse.bass as bass
import concourse.mybir as mybir
import concourse.tile as tile
from concourse.masks import make_identity

F32 = mybir.dt.float32
BF16 = mybir.dt.bfloat16
AF = mybir.ActivationFunctionType
ALU = mybir.AluOpType
AX = mybir.AxisListType

B, N_HOP, N_MEM, DIM = 1024, 2, 64, 32
N_CORES = 8
BC = B // N_CORES            # 128 batches per core
NBLK = 16                    # blocks of 8 batches
NG = 4                       # g per bp
D4 = 4 * DIM                 # 128 MLP input features
RB = 4                       # blocks per R DMA
NGRP = NBLK // RB            # 4 groups per hop
ROWS = 512                   # zt columns per block (4 g-chunks x 128)

R_QUEUES = ("sync", "scalar")    # rotate r-tile DMA trigger queues
GPSIMD_MUL = True                # odd groups' R*h mul on Pool engine
ABS_ON_DVE = True                # |x| via tensor_scalar abs_max on DVE


def build_nc(n_iter: int = 1) -> bass.Bass:
    nc = bacc.Bacc("TRN2")

    Rs_d = nc.dram_tensor(
        "Rs", [N_HOP, NBLK, 2, N_MEM, NG, DIM, DIM], BF16, kind="ExternalInput"
    )
    hs_d = nc.dram_tensor(
        "hs", [N_HOP, 2, N_MEM, NBLK, NG, DIM], BF16, kind="ExternalInput"
    )
    ts_d = nc.dram_tensor("ts", [BC, N_HOP, DIM, N_MEM], BF16, kind="ExternalInput")
    vrep_d = nc.dram_tensor(
        "v_rep", [128, NBLK * NG * DIM], BF16, kind="ExternalInput"
    )
    vs_d = nc.dram_tensor("vs", [BC, DIM], F32, kind="ExternalInput")
    W1T_d = nc.dram_tensor("W1T", [D4, DIM], BF16, kind="ExternalInput")
    W2T_d = nc.dram_tensor("W2T", [DIM, 1], BF16, kind="ExternalInput")
    b1_d = nc.dram_tensor("b1", [DIM], F32, kind="ExternalInput")
    WihT_d = nc.dram_tensor("WihT", [N_HOP, DIM, 3 * DIM], F32, kind="ExternalInput")
    WhhT_d = nc.dram_tensor("WhhT", [N_HOP, DIM, 3 * DIM], F32, kind="ExternalInput")
    bih_d = nc.dram_tensor("b_ih", [N_HOP, 3 * DIM], F32, kind="ExternalInput")
    bhh_d = nc.dram_tensor("b_hh", [N_HOP, 3 * DIM], F32, kind="ExternalInput")
    out_d = nc.dram_tensor("out", [BC, DIM], F32, kind="ExternalOutput")
    m_scr = nc.dram_tensor("m_scratch", [BC, DIM], BF16)
    m_scr2 = nc.dram_tensor("m_scratch2", [2, NBLK, NG, DIM], BF16)
    m_scr3 = nc.dram_tensor("m_scratch3", [128, NBLK * NG * DIM], BF16)
    z_scr = nc.dram_tensor("z_scratch", [NBLK, 2, NG, N_MEM], BF16)

    import contextlib

    with tile.TileContext(nc) as tc:
        with (
            (tc.For_i(0, n_iter, 1) if n_iter > 1 else contextlib.nullcontext()),
            tc.tile_pool(name="consts", bufs=1) as consts,
            tc.tile_pool(name="hop_io", bufs=2) as hop_io,
            tc.tile_pool(name="rpool", bufs=3) as rpool,
            tc.tile_pool(name="fpool", bufs=3) as fpool,
            tc.tile_pool(name="zpool", bufs=3) as zpool,
            tc.tile_pool(name="apool", bufs=3) as apool,
            tc.tile_pool(name="small", bufs=2) as small,
            tc.tile_pool(name="mstate", bufs=2) as mstate,
            tc.tile_pool(name="pp_z", bufs=2, space="PSUM") as pp_z,
            tc.tile_pool(name="pp_1", bufs=2, space="PSUM") as pp_1,
            tc.tile_pool(name="pp_2", bufs=2, space="PSUM") as pp_2,
            tc.tile_pool(name="pp_g", bufs=2, space="PSUM") as pp_g,
        ):
            ident = consts.tile([128, 128], F32)
            make_identity(nc, ident)
            ident_bf = consts.tile([128, 128], BF16)
            make_identity(nc, ident_bf)

            # preload h/t for both hops (Act queue) so the hop-1 einsum never
            # stalls on them behind hop-0's z writes
            h_hops, t_hops = [], []
            for hop in range(N_HOP):
                h_hop = hop_io.tile([128, NBLK * NG * DIM], BF16, tag="h_hop")
                nc.scalar.dma_start(
                    out=h_hop,
                    in_=hs_d[hop].rearrange("bp m blk g e -> (bp m) (blk g e)"),
                )
                h_hops.append(h_hop)
                # t d-major [b, (d m)] so the m-reduce is innermost-packed
                t_hop = hop_io.tile([BC, DIM * N_MEM], BF16, tag="t_hop")
                nc.scalar.dma_start(
                    out=t_hop, in_=ts_d[:, hop].rearrange("b d m -> b (d m)")
                )
                t_hops.append(t_hop)

            # ---- weights (host-pretransposed, direct loads) ----
            W1T = consts.tile([D4, DIM], BF16)
            nc.scalar.dma_start(out=W1T, in_=W1T_d[:, :])
            W2T = consts.tile([DIM, 1], BF16)
            nc.scalar.dma_start(out=W2T, in_=W2T_d[:, :])
            b1T = consts.tile([DIM, 1], F32)
            nc.scalar.dma_start(out=b1T, in_=b1_d[:].unsqueeze(1))

            WihT, WhhT, bsum_rz, bihn_t, bhhn_t = [], [], [], [], []
            for hop in range(N_HOP):
                wT = consts.tile([DIM, 3 * DIM], F32, tag=f"wihT{hop}")
                nc.scalar.dma_start(out=wT, in_=WihT_d[hop])
                WihT.append(wT)
                wT2 = consts.tile([DIM, 3 * DIM], F32, tag=f"whhT{hop}")
                nc.scalar.dma_start(out=wT2, in_=WhhT_d[hop])
                WhhT.append(wT2)

                gate_b = []
                for gd, gname in ((bih_d, "ih"), (bhh_d, "hh")):
                    for gate in range(3):
                        bt = consts.tile([DIM, 1], F32, tag=f"b{gname}{hop}{gate}")
                        nc.scalar.dma_start(
                            out=bt,
                            in_=gd[hop, gate * DIM : (gate + 1) * DIM].unsqueeze(1),
                        )
                        gate_b.append(bt)
                b_r = consts.tile([DIM, 1], F32, tag=f"b_r{hop}")
                nc.vector.tensor_add(b_r, gate_b[0], gate_b[3])
                b_z = consts.tile([DIM, 1], F32, tag=f"b_z{hop}")
                nc.vector.tensor_add(b_z, gate_b[1], gate_b[4])
                bsum_rz.append((b_r, b_z))
                bihn_t.append(gate_b[2])
                bhhn_t.append(gate_b[5])

            # ---- initial M state ----
            vs_row = consts.tile([BC, DIM], F32)
            nc.scalar.dma_start(out=vs_row, in_=vs_d[:, :])
            vst_ps = pp_g.tile([DIM, BC], F32, tag="gpsum")
            nc.tensor.transpose(vst_ps, vs_row, ident)
            vsT = consts.tile([DIM, BC], F32)
            nc.scalar.copy(out=vsT, in_=vst_ps)
            MT = vsT  # current M^T [d, b]

            v_rep = consts.tile([128, NBLK * NG * DIM], BF16)
            nc.scalar.dma_start(out=v_rep, in_=vrep_d[:, :])
            M_rep = v_rep  # hop 0: M == vs

            for hop in range(N_HOP):
                h_hop = h_hops[hop]
                t_hop = t_hops[hop]
                deferred_z = []

                for grp in range(NGRP):
                    blk0 = grp * RB
                    r_tile = rpool.tile([128, RB * NG * DIM * DIM], BF16, tag="R")
                    # SP carries ONLY the R stream: its in-order SEQ must
                    # never wait on downstream compute
                    nc.sync.dma_start(
                        out=r_tile.rearrange("p (blk f) -> p blk f", blk=RB),
                        in_=Rs_d[hop, blk0 : blk0 + RB].rearrange(
                            "blk bp m g d e -> (bp m) blk (g d e)"
                        ),
                    )
                    # P = R * h (in-place), h broadcast over d
                    r5 = r_tile.rearrange(
                        "p (blk g d e) -> p blk g d e", blk=RB, g=NG, d=DIM
                    )
                    h_v = (
                        h_hop[:, blk0 * NG * DIM : (blk0 + RB) * NG * DIM]
                        .rearrange("p (blk g e) -> p blk g e", blk=RB, g=NG)
                        .unsqueeze(3)
                        .broadcast_to((128, RB, NG, DIM, DIM))
                    )
                    # HW-calibrated split: Pool runs TT at ~1.45 ns/elem vs
                    # DVE 0.54, so Pool only takes every other group's mul;
                    # trees+features stay on DVE.
                    if grp % 2 == 1:
                        nc.gpsimd.tensor_tensor(r5, r5, h_v, op=ALU.mult)
                    else:
                        nc.vector.tensor_mul(r5, r5, h_v)
                    # Rh[(bp,m), (blk,g,d)] = sum_e P via in-place halving
                    # tree: TensorTensor adds run at 2 elem/cycle on DVE vs
                    # TensorReduce's 1.
                    rh = fpool.tile([128, RB * NG * DIM], BF16, tag="rh")
                    v3 = r_tile.rearrange("p (gd e) -> p gd e", e=DIM)
                    for w in (16, 8, 4, 2):
                        nc.vector.tensor_add(
                            v3[:, :, 0:w], v3[:, :, 0:w], v3[:, :, w : 2 * w]
                        )
                    nc.vector.tensor_add(rh, v3[:, :, 0], v3[:, :, 1])
                    # features F [(bp,m), (blk, g, f, d)]
                    f_grp = fpool.tile([128, RB * NG * 4 * DIM], BF16, tag="F")
                    f5 = f_grp.rearrange(
                        "p (blk g f d) -> p blk g f d", blk=RB, g=NG, f=4
                    )
                    rh4 = rh.rearrange("p (blk g d) -> p blk g d", blk=RB, g=NG)
                    vr4 = v_rep[
                        :, blk0 * NG * DIM : (blk0 + RB) * NG * DIM
                    ].rearrange("p (blk g d) -> p blk g d", blk=RB, g=NG)
                    mr4 = M_rep[
                        :, blk0 * NG * DIM : (blk0 + RB) * NG * DIM
                    ].rearrange("p (blk g d) -> p blk g d", blk=RB, g=NG)
                    feng = nc.vector
                    feng.tensor_tensor(f5[:, :, :, 0, :], rh4, vr4, op=ALU.mult)
                    feng.tensor_tensor(f5[:, :, :, 1, :], rh4, mr4, op=ALU.mult)
                    feng.tensor_tensor(f5[:, :, :, 2, :], rh4, vr4, op=ALU.subtract)
                    feng.tensor_tensor(f5[:, :, :, 3, :], rh4, mr4, op=ALU.subtract)
                    nc.scalar.activation(
                        f5[:, :, :, 2, :], f5[:, :, :, 2, :], AF.Abs
                    )
                    nc.scalar.activation(
                        f5[:, :, :, 3, :], f5[:, :, :, 3, :], AF.Abs
                    )

                    for pair in range(RB // 2):
                        # transpose 2 blocks into one PSUM bank ([128, 1024]
                        # bf16 = 2 KB/partition) -> single Act copy-out
                        zt_ps = pp_z.tile([D4, 2 * ROWS], BF16, tag="zt")
                        for j in range(2):
                            blk = pair * 2 + j
                            for g in range(NG):
                                c0 = (blk * NG + g) * 128
                                nc.tensor.transpose(
                                    zt_ps[
                                        :, j * ROWS + g * 128 : j * ROWS + (g + 1) * 128
                                    ],
                                    f_grp[:, c0 : c0 + 128],
                                    ident_bf,
                                )
                        zt_sb = zpool.tile([D4, 2 * ROWS], BF16, tag="zt_sb")
                        nc.scalar.copy(out=zt_sb, in_=zt_ps)

                        for j in range(2):
                            blk = blk0 + pair * 2 + j
                            ps1 = pp_1.tile([DIM, ROWS], F32, tag="ps1")
                            nc.tensor.matmul(
                                ps1,
                                lhsT=W1T,
                                rhs=zt_sb[:, j * ROWS : (j + 1) * ROWS],
                                start=True,
                                stop=True,
                            )
                            a1 = apool.tile([DIM, ROWS], BF16, tag="a1")
                            nc.scalar.activation(a1, ps1, AF.Tanh, bias=b1T)
                            ps2 = pp_2.tile([1, ROWS], F32, tag="ps2")
                            nc.tensor.matmul(
                                ps2, lhsT=W2T, rhs=a1, start=True, stop=True
                            )
                            z_sb = zpool.tile(
                                [1, ROWS], BF16, tag=f"z_sb{hop}",
                                bufs=(NBLK if hop == N_HOP - 1 else 3),
                            )
                            nc.scalar.copy(out=z_sb, in_=ps2)
                            # z_sb free order (g, bp, m); store as (bp, g, m)
                            # so the gather's partition dim is adjacent.
                            # hop 0: write now on Act. last hop: defer to SP
                            # (after its R stream) so Act is not the serializer.
                            if hop < N_HOP - 1:
                                nc.scalar.dma_start(
                                    out=z_scr[blk]
                                    .unsqueeze(0)
                                    .rearrange("o bp g m -> o g bp m"),
                                    in_=z_sb.rearrange(
                                        "o (g bp m) -> o g bp m", g=NG, bp=2
                                    ),
                                )
                            else:
                                deferred_z.append((blk, z_sb))

                if hop == N_HOP - 1:
                    for blk, z_sb in deferred_z:
                        nc.sync.dma_start(
                            out=z_scr[blk]
                            .unsqueeze(0)
                            .rearrange("o bp g m -> o g bp m"),
                            in_=z_sb.rearrange(
                                "o (g bp m) -> o g bp m", g=NG, bp=2
                            ),
                        )

                # gather Z rows: flat (blk,bp,g) == natural b, m packed
                Z_row = small.tile([BC, N_MEM], BF16, tag="Z_row")
                zq = nc.sync if hop == N_HOP - 1 else nc.scalar
                zq.dma_start(
                    out=Z_row,
                    in_=z_scr.rearrange("blk bp g m -> (blk bp g) m"),
                )

                # softmax over m, batched over all 128 b
                nmx = small.tile([BC, 1], F32, tag="nmx")
                nc.vector.tensor_reduce(
                    out=nmx, in_=Z_row, axis=AX.X, op=ALU.max, negate=True
                )
                e_row = small.tile([BC, N_MEM], F32, tag="e_row")
                nc.scalar.activation(e_row, Z_row, AF.Exp, bias=nmx)
                ssum = small.tile([BC, 1], F32, tag="ssum")
                nc.vector.tensor_reduce(out=ssum, in_=e_row, axis=AX.X, op=ALU.add)
                rsum = small.tile([BC, 1], F32, tag="rsum")
                nc.vector.reciprocal(rsum, ssum)
                g_bf = small.tile([BC, N_MEM], BF16, tag="g_bf")
                nc.vector.tensor_scalar_mul(g_bf, e_row, rsum)

                # o[b,d] = sum_m t[b,d,m] * g[b,m]  (in-place mul into t_hop)
                t3 = t_hop.rearrange("b (d m) -> b d m", d=DIM)
                g3 = g_bf.unsqueeze(1).broadcast_to((BC, DIM, N_MEM))
                nc.vector.tensor_mul(t3, t3, g3)
                for w in (32, 16, 8, 4, 2):
                    nc.vector.tensor_add(
                        t3[:, :, 0:w], t3[:, :, 0:w], t3[:, :, w : 2 * w]
                    )
                o_row = small.tile([BC, DIM], F32, tag="o_row")
                nc.vector.tensor_add(o_row, t3[:, :, 0], t3[:, :, 1])

                # GRU (transposed layout [*, b], fp32)
                ot_ps = pp_g.tile([DIM, BC], F32, tag="gpsum")
                nc.tensor.transpose(ot_ps, o_row, ident)
                oT = small.tile([DIM, BC], F32, tag="oT")
                nc.scalar.copy(out=oT, in_=ot_ps)

                def gate_pair(g):
                    gi = pp_g.tile([DIM, BC], F32, tag="gpsum")
                    nc.tensor.matmul(
                        gi,
                        lhsT=WihT[hop][:, g * DIM : (g + 1) * DIM],
                        rhs=oT,
                        start=True,
                        stop=True,
                    )
                    gh = pp_g.tile([DIM, BC], F32, tag="gpsum")
                    nc.tensor.matmul(
                        gh,
                        lhsT=WhhT[hop][:, g * DIM : (g + 1) * DIM],
                        rhs=MT,
                        start=True,
                        stop=True,
                    )
                    return gi, gh

                rz_t = []
                for g in range(2):
                    gi, gh = gate_pair(g)
                    gb = small.tile([DIM, BC], F32, tag=f"g{g}b")
                    nc.vector.tensor_scalar(
                        out=gb, in0=gi, scalar1=bsum_rz[hop][g], scalar2=None,
                        op0=ALU.add,
                    )
                    nc.vector.tensor_add(gb, gb, gh)
                    gt = small.tile([DIM, BC], F32, tag=f"gate{g}")
                    # sigmoid via tanh keeps every Act func in the
                    # exp_and_others table: no LoadActFuncSet swaps
                    nc.scalar.activation(gt, gb, AF.Tanh, scale=0.5)
                    nc.vector.tensor_scalar(
                        out=gt, in0=gt, scalar1=0.5, scalar2=0.5,
                        op0=ALU.mult, op1=ALU.add,
                    )
                    rz_t.append(gt)
                r_t, z_t = rz_t

                gi_n, gh_n = gate_pair(2)
                ghn = small.tile([DIM, BC], F32, tag="ghn")
                nc.vector.tensor_scalar(
                    out=ghn, in0=gh_n, scalar1=bhhn_t[hop], scalar2=None,
                    op0=ALU.add,
                )
                gin = small.tile([DIM, BC], F32, tag="gin")
                nc.vector.tensor_scalar(
                    out=gin, in0=gi_n, scalar1=bihn_t[hop], scalar2=None,
                    op0=ALU.add,
                )
                n1 = small.tile([DIM, BC], F32, tag="n1")
                nc.vector.tensor_mul(n1, r_t, ghn)
                nc.vector.tensor_add(n1, n1, gin)
                n_t = small.tile([DIM, BC], F32, tag="n_t")
                nc.scalar.activation(n_t, n1, AF.Tanh)

                # M' = n + z * (M - n)
                MT_new = mstate.tile([DIM, BC], F32, tag="MT")
                nc.vector.tensor_sub(MT_new, MT, n_t)
                nc.vector.tensor_mul(MT_new, MT_new, z_t)
                nc.vector.tensor_add(MT_new, MT_new, n_t)
                MT = MT_new

                # M_row for output / M_rep rebuild
                mrow_ps = pp_g.tile([BC, DIM], F32, tag="gpsum")
                nc.tensor.transpose(mrow_ps, MT, ident[:DIM, :DIM])
                M_row = mstate.tile([BC, DIM], F32, tag="M_row")
                nc.scalar.copy(out=M_row, in_=mrow_ps)

                if hop < N_HOP - 1:
                    # rebuild M_rep (bf16) via DRAM bounce
                    Mb = mstate.tile([BC, DIM], BF16, tag="Mb")
                    nc.scalar.copy(out=Mb, in_=M_row)
                    nc.scalar.dma_start(out=m_scr[:, :], in_=Mb)
                    # DRAM->DRAM permute: natural b -> [bp, blk, g, d]
                    nc.scalar.dma_start(
                        out=m_scr2[:, :, :, :],
                        in_=m_scr.rearrange(
                            "(blk bp g) d -> bp blk g d", bp=2, g=NG
                        ),
                    )
                    # replicate rows to (bp, m) partitions in DRAM
                    # partition_broadcast prepends the broadcast dim: in is
                    # [m(bcast), bp, f]; write rows r = bp*64+m accordingly
                    nc.scalar.dma_start(
                        out=m_scr3.rearrange("(bp m) f -> m bp f", bp=2),
                        in_=m_scr2.rearrange(
                            "bp blk g d -> bp (blk g d)"
                        ).partition_broadcast(N_MEM),
                    )
                    M_rep_new = mstate.tile(
                        [128, NBLK * NG * DIM], BF16, tag="M_rep", bufs=1
                    )
                    nc.scalar.dma_start(out=M_rep_new, in_=m_scr3[:, :])
                    M_rep = M_rep_new
                else:
                    nc.sync.dma_start(out=out_d[:, :], in_=M_row)

    nc.compile()
    return nc


_NC_CACHE = None


def _get_nc():
    global _NC_CACHE
    if _NC_CACHE is None:
        _NC_CACHE = build_nc()
    return _NC_CACHE


BF_NP = ml_dtypes.bfloat16


def make_in_maps(hs, Rs, ts, vs, W1, b1, W2, W_ih, W_hh, b_ih, b_hh):
    hs = np.asarray(hs)
    Rs = np.asarray(Rs)
    ts = np.asarray(ts)
    vs = np.asarray(vs)
    W1T = np.ascontiguousarray(np.asarray(W1).T).astype(BF_NP)
    W2T = np.ascontiguousarray(np.asarray(W2).T).astype(BF_NP)
    b1 = np.ascontiguousarray(np.asarray(b1))
    WihT = np.ascontiguousarray(np.asarray(W_ih).transpose(0, 2, 1))
    WhhT = np.ascontiguousarray(np.asarray(W_hh).transpose(0, 2, 1))
    b_ih = np.ascontiguousarray(np.asarray(b_ih))
    b_hh = np.ascontiguousarray(np.asarray(b_hh))

    in_maps = []
    for c in range(N_CORES):
        sl = slice(c * BC, (c + 1) * BC)
        # Rs [BC, hop, m, d, e] -> [hop, blk, bp, m, g, d, e]
        Rp = (
            Rs[sl]
            .reshape(NBLK, 2, NG, N_HOP, N_MEM, DIM, DIM)
            .transpose(3, 0, 1, 4, 2, 5, 6)
            .astype(BF_NP)
        )
        # hs [BC, hop, m, e] -> [hop, bp, m, blk, g, e]
        hp = (
            hs[sl]
            .reshape(NBLK, 2, NG, N_HOP, N_MEM, DIM)
            .transpose(3, 1, 4, 0, 2, 5)
            .astype(BF_NP)
        )
        # ts [BC, hop, m, d] -> [BC, hop, d, m]
        tp = ts[sl].transpose(0, 1, 3, 2).astype(BF_NP)
        vsc = np.ascontiguousarray(vs[sl])
        # v_rep [128=(bp,m), (blk, g, d)]
        vv = vsc.reshape(NBLK, 2, NG, DIM).transpose(1, 0, 2, 3)  # [bp, blk, g, d]
        v_rep = (
            np.broadcast_to(vv[:, None], (2, N_MEM, NBLK, NG, DIM))
            .reshape(128, NBLK * NG * DIM)
            .astype(BF_NP)
        )
        in_maps.append(
            {
                "Rs": np.ascontiguousarray(Rp),
                "hs": np.ascontiguousarray(hp),
                "ts": np.ascontiguousarray(tp),
                "v_rep": np.ascontiguousarray(v_rep),
                "vs": vsc,
                "W1T": W1T,
                "W2T": W2T,
                "b1": b1,
                "WihT": WihT,
                "WhhT": WhhT,
                "b_ih": b_ih,
                "b_hh": b_hh,
            }
        )
    return in_maps


def kernel(hs, Rs, ts, vs, W1, b1, W2, b2, W_ih, W_hh, b_ih, b_hh):
    from concourse.bass_utils import run_bass_kernel_spmd

    nc = _get_nc()
    in_maps = make_in_maps(hs, Rs, ts, vs, W1, b1, W2, W_ih, W_hh, b_ih, b_hh)
    res = run_bass_kernel_spmd(nc, in_maps, list(range(N_CORES)))
    return np.concatenate([r["out"] for r in res.results], axis=0)


# revision 28
# speedup vs baseline: 1.3256x; 1.0051x over previous
"""EpisodicMemory kernel for Trainium2, 8-core data-parallel, bf16 compute.

Reference computation (per batch b, d=32, m=64 memory slots, 2 hops):
    M = vs[b]
    for hop:
        Rh[m,:] = R[b,hop,m] @ h[b,hop,m]                  # batched matvec
        z = [Rh*v, Rh*M, |Rh-v|, |Rh-M|]                   # [m, 4d]
        Z = tanh(z @ W1.T + b1) @ W2.T (+ b2: dropped — softmax-invariant)
        g = softmax(Z over m); o = sum_m ts[b,hop,m] * g[m]
        M = GRUCell(o, M)
    out[b] = M

Sharding: pure data parallel over batch; 128 batches per core.

bf16 strategy (tolerance 2e-2; measured end-to-end rel err ~1.6e-3):
  Rs/hs/ts host-converted to bf16 — halves the dominant HBM traffic
  (Rs 64 MiB -> 32 MiB per core). Einsum product in bf16; reduction via
  an in-place halving tree of TensorTensor adds (DVE runs TT at ~0.54
  ns/elem vs TensorReduce ~1.04). MLP matmuls bf16 (PSUM fp32), softmax
  stats fp32, GRU fp32.

HW-calibrated engine split (microbenched): DVE 0.54 ns/elem, Pool 1.45
ns/elem, SP DMA ring ~600 GB/s for 4 MiB transfers. Pool takes every
other group's R*h mul; DVE does the other muls + all trees + features.
Sigmoid is computed as 0.5+0.5*tanh(x/2) so every Act function stays in
the one exp_and_others table (no LoadActFuncSet swaps).

Queue discipline: the SP SEQ is in-order and blocks on each DMA, so SP
carries ONLY the R stream (plus last-hop z flush + out after it). All
other traffic (h/t, packed consts, hop-0 z, gathers, M bounce) rides the
Act queue. Weights arrive packed in two tensors (one DMA each); z rows
are collected in one SBUF tile per hop and flushed with 2 DMAs.

Per-core layout: b = blk*8 + bp*4 + g; partition p = bp*64 + m.
  - R tiles [128 part=(bp,m), free=(blk4,g4,d32,e32)]: 4 blocks per DMA.
  - z_scr DRAM [blk, bp, g, m]: gather = one clean [128, 64] load.
  - v_rep/M_rep [128, (blk g d)] bf16; v_rep host-prebuilt, M_rep via
    4-DMA DRAM bounce after hop 0.
"""

import numpy as np
import ml_dtypes

import concourse.bacc as bacc
import concourse.bass as bass
import concourse.mybir as mybir
import concourse.tile as tile
from concourse.masks import make_identity

F32 = mybir.dt.float32
BF16 = mybir.dt.bfloat16
AF = mybir.ActivationFunctionType
ALU = mybir.AluOpType
AX = mybir.AxisListType

B, N_HOP, N_MEM, DIM = 1024, 2, 64, 32
N_CORES = 8
BC = B // N_CORES            # 128 batches per core
NBLK = 16                    # blocks of 8 batches
NG = 4                       # g per bp
D4 = 4 * DIM                 # 128 MLP input features
RB = 4                       # blocks per R DMA
NGRP = NBLK // RB            # 4 groups per hop
ROWS = 512                   # zt columns per block (4 g-chunks x 128)

# packed fp32 consts layout (columns)
PF_VS = 0                    # [128, 0:32]   vs rows
PF_B1 = 32                   # [0:32, 32]    b1
PF_W = 33                    # [0:32, 33+]   WihT0 WhhT0 WihT1 WhhT1 (96 each)
PF_BIAS = PF_W + 4 * 96      # [0:32, 417+]  bih0(3) bhh0(3) bih1(3) bhh1(3)
PF_COLS = PF_BIAS + 12       # 429


def build_nc(n_iter: int = 1) -> bass.Bass:
    nc = bacc.Bacc("TRN2")

    Rs_d = nc.dram_tensor(
        "Rs", [N_HOP, NBLK, 2, N_MEM, NG, DIM, DIM], BF16, kind="ExternalInput"
    )
    hs_d = nc.dram_tensor(
        "hs", [N_HOP, 2, N_MEM, NBLK, NG, DIM], BF16, kind="ExternalInput"
    )
    ts_d = nc.dram_tensor("ts", [BC, N_HOP, DIM, N_MEM], BF16, kind="ExternalInput")
    vrep_d = nc.dram_tensor(
        "v_rep", [128, NBLK * NG * DIM], BF16, kind="ExternalInput"
    )
    pkbf_d = nc.dram_tensor("pk_bf", [128, 33], BF16, kind="ExternalInput")
    pkf_d = nc.dram_tensor("pk_f32", [128, PF_COLS], F32, kind="ExternalInput")
    out_d = nc.dram_tensor("out", [BC, DIM], F32, kind="ExternalOutput")
    m_scr = nc.dram_tensor("m_scratch", [BC, DIM], BF16)
    m_scr2 = nc.dram_tensor("m_scratch2", [2, NBLK, NG, DIM], BF16)
    m_scr3 = nc.dram_tensor("m_scratch3", [128, NBLK * NG * DIM], BF16)
    z_scr = nc.dram_tensor("z_scratch", [NBLK, 2, NG, N_MEM], BF16)

    import contextlib

    with tile.TileContext(nc) as tc:
        with (
            (tc.For_i(0, n_iter, 1) if n_iter > 1 else contextlib.nullcontext()),
            tc.tile_pool(name="consts", bufs=1) as consts,
            tc.tile_pool(name="hop_io", bufs=2) as hop_io,
            tc.tile_pool(name="rpool", bufs=3) as rpool,
            tc.tile_pool(name="fpool", bufs=3) as fpool,
            tc.tile_pool(name="zpool", bufs=3) as zpool,
            tc.tile_pool(name="apool", bufs=3) as apool,
            tc.tile_pool(name="small", bufs=2) as small,
            tc.tile_pool(name="mstate", bufs=2) as mstate,
            tc.tile_pool(name="pp_z", bufs=2, space="PSUM") as pp_z,
            tc.tile_pool(name="pp_1", bufs=2, space="PSUM") as pp_1,
            tc.tile_pool(name="pp_2", bufs=2, space="PSUM") as pp_2,
            tc.tile_pool(name="pp_g", bufs=2, space="PSUM") as pp_g,
        ):
            ident = consts.tile([128, 128], F32)
            make_identity(nc, ident)
            ident_bf = consts.tile([128, 128], BF16)
            make_identity(nc, ident_bf)

            # preload h/t for both hops (Act queue) so the hop-1 einsum never
            # stalls on them behind hop-0's z writes
            h_hops, t_hops = [], []
            for hop in range(N_HOP):
                h_hop = hop_io.tile([128, NBLK * NG * DIM], BF16, tag="h_hop")
                nc.scalar.dma_start(
                    out=h_hop,
                    in_=hs_d[hop].rearrange("bp m blk g e -> (bp m) (blk g e)"),
                )
                h_hops.append(h_hop)
                # t d-major [b, (d m)] so the m-reduce is innermost-packed
                t_hop = hop_io.tile([BC, DIM * N_MEM], BF16, tag="t_hop")
                nc.scalar.dma_start(
                    out=t_hop, in_=ts_d[:, hop].rearrange("b d m -> b (d m)")
                )
                t_hops.append(t_hop)

            v_rep = consts.tile([128, NBLK * NG * DIM], BF16)
            nc.scalar.dma_start(out=v_rep, in_=vrep_d[:, :])
            M_rep = v_rep  # hop 0: M == vs

            # ---- packed weights: 2 DMAs total ----
            pk_bf = consts.tile([128, 33], BF16)
            nc.scalar.dma_start(out=pk_bf, in_=pkbf_d[:, :])
            pk_f = consts.tile([128, PF_COLS], F32)
            nc.scalar.dma_start(out=pk_f, in_=pkf_d[:, :])

            W1T = pk_bf[:, 0:32]
            W2T = pk_bf[0:DIM, 32:33]
            b1T = pk_f[0:DIM, PF_B1 : PF_B1 + 1]
            WihT = [pk_f[0:DIM, PF_W + 192 * h : PF_W + 192 * h + 96]
                    for h in range(N_HOP)]
            WhhT = [pk_f[0:DIM, PF_W + 192 * h + 96 : PF_W + 192 * h + 192]
                    for h in range(N_HOP)]

            bsum_rz, bihn_t, bhhn_t = [], [], []
            for hop in range(N_HOP):
                c0 = PF_BIAS + 6 * hop
                gate_b = [pk_f[0:DIM, c0 + j : c0 + j + 1] for j in range(6)]
                b_r = consts.tile([DIM, 1], F32, tag=f"b_r{hop}")
                nc.vector.tensor_add(b_r, gate_b[0], gate_b[3])
                b_z = consts.tile([DIM, 1], F32, tag=f"b_z{hop}")
                nc.vector.tensor_add(b_z, gate_b[1], gate_b[4])
                bsum_rz.append((b_r, b_z))
                bihn_t.append(gate_b[2])
                bhhn_t.append(gate_b[5])

            # ---- initial M state ----
            vs_row = pk_f[:, PF_VS : PF_VS + 32]
            vst_ps = pp_g.tile([DIM, BC], F32, tag="gpsum")
            nc.tensor.transpose(vst_ps, vs_row, ident)
            vsT = consts.tile([DIM, BC], F32)
            nc.scalar.copy(out=vsT, in_=vst_ps)
            MT = vsT  # current M^T [d, b]

            for hop in range(N_HOP):
                h_hop = h_hops[hop]
                t_hop = t_hops[hop]
                # per-hop z collector [1, (blk, g, bp, m)]
                z_all = zpool.tile(
                    [1, NBLK * ROWS], BF16, tag=f"z_all{hop}", bufs=1
                )

                for grp in range(NGRP):
                    blk0 = grp * RB
                    r_tile = rpool.tile([128, RB * NG * DIM * DIM], BF16, tag="R")
                    # SP carries ONLY the R stream: its in-order SEQ must
                    # never wait on downstream compute
                    nc.sync.dma_start(
                        out=r_tile.rearrange("p (blk f) -> p blk f", blk=RB),
                        in_=Rs_d[hop, blk0 : blk0 + RB].rearrange(
                            "blk bp m g d e -> (bp m) blk (g d e)"
                        ),
                    )
                    # P = R * h (in-place), h broadcast over d
                    r5 = r_tile.rearrange(
                        "p (blk g d e) -> p blk g d e", blk=RB, g=NG, d=DIM
                    )
                    h_v = (
                        h_hop[:, blk0 * NG * DIM : (blk0 + RB) * NG * DIM]
                        .rearrange("p (blk g e) -> p blk g e", blk=RB, g=NG)
                        .unsqueeze(3)
                        .broadcast_to((128, RB, NG, DIM, DIM))
                    )
                    # HW-calibrated split: Pool runs TT at ~1.45 ns/elem vs
                    # DVE 0.54, so Pool only takes every other group's mul
                    if grp % 2 == 1:
                        nc.gpsimd.tensor_tensor(r5, r5, h_v, op=ALU.mult)
                    else:
                        nc.vector.tensor_mul(r5, r5, h_v)
                    # Rh[(bp,m), (blk,g,d)] = sum_e P via in-place halving
                    # tree (TT adds at 2 elem/cycle vs TensorReduce's 1)
                    rh = fpool.tile([128, RB * NG * DIM], BF16, tag="rh")
                    v3 = r_tile.rearrange("p (gd e) -> p gd e", e=DIM)
                    for w in (16, 8, 4, 2):
                        nc.vector.tensor_add(
                            v3[:, :, 0:w], v3[:, :, 0:w], v3[:, :, w : 2 * w]
                        )
                    nc.vector.tensor_add(rh, v3[:, :, 0], v3[:, :, 1])
                    # features F [(bp,m), (blk, g, f, d)]
                    f_grp = fpool.tile([128, RB * NG * 4 * DIM], BF16, tag="F")
                    f5 = f_grp.rearrange(
                        "p (blk g f d) -> p blk g f d", blk=RB, g=NG, f=4
                    )
                    rh4 = rh.rearrange("p (blk g d) -> p blk g d", blk=RB, g=NG)
                    vr4 = v_rep[
                        :, blk0 * NG * DIM : (blk0 + RB) * NG * DIM
                    ].rearrange("p (blk g d) -> p blk g d", blk=RB, g=NG)
                    mr4 = M_rep[
                        :, blk0 * NG * DIM : (blk0 + RB) * NG * DIM
                    ].rearrange("p (blk g d) -> p blk g d", blk=RB, g=NG)
                    nc.vector.tensor_mul(f5[:, :, :, 0, :], rh4, vr4)
                    nc.vector.tensor_mul(f5[:, :, :, 1, :], rh4, mr4)
                    nc.vector.tensor_sub(f5[:, :, :, 2, :], rh4, vr4)
                    nc.vector.tensor_sub(f5[:, :, :, 3, :], rh4, mr4)
                    nc.scalar.activation(
                        f5[:, :, :, 2, :], f5[:, :, :, 2, :], AF.Abs
                    )
                    nc.scalar.activation(
                        f5[:, :, :, 3, :], f5[:, :, :, 3, :], AF.Abs
                    )

                    for pair in range(RB // 2):
                        # transpose 2 blocks into one PSUM bank ([128, 1024]
                        # bf16 = 2 KB/partition) -> single Act copy-out
                        zt_ps = pp_z.tile([D4, 2 * ROWS], BF16, tag="zt")
                        for j in range(2):
                            blk = pair * 2 + j
                            for g in range(NG):
                                c0 = (blk * NG + g) * 128
                                nc.tensor.transpose(
                                    zt_ps[
                                        :, j * ROWS + g * 128 : j * ROWS + (g + 1) * 128
                                    ],
                                    f_grp[:, c0 : c0 + 128],
                                    ident_bf,
                                )
                        zt_sb = zpool.tile([D4, 2 * ROWS], BF16, tag="zt_sb")
                        nc.scalar.copy(out=zt_sb, in_=zt_ps)

                        for j in range(2):
                            blk = blk0 + pair * 2 + j
                            ps1 = pp_1.tile([DIM, ROWS], F32, tag="ps1")
                            nc.tensor.matmul(
                                ps1,
                                lhsT=W1T,
                                rhs=zt_sb[:, j * ROWS : (j + 1) * ROWS],
                                start=True,
                                stop=True,
                            )
                            a1 = apool.tile([DIM, ROWS], BF16, tag="a1")
                            nc.scalar.activation(a1, ps1, AF.Tanh, bias=b1T)
                            ps2 = pp_2.tile([1, ROWS], F32, tag="ps2")
                            nc.tensor.matmul(
                                ps2, lhsT=W2T, rhs=a1, start=True, stop=True
                            )
                            # collect z into the per-hop SBUF tile; flushed
                            # with 2 DMAs at hop end
                            nc.scalar.copy(
                                out=z_all[:, blk * ROWS : (blk + 1) * ROWS],
                                in_=ps2,
                            )

                # flush z: z_all free order (blk, g, bp, m) -> z_scr
                # [blk, bp, g, m]; one DMA per bp keeps both APs at 3 dims
                zq = nc.sync if hop == N_HOP - 1 else nc.scalar
                z5 = z_all.rearrange(
                    "o (blk g bp m) -> o blk g bp m", blk=NBLK, g=NG, bp=2
                )
                for bp in range(2):
                    zq.dma_start(
                        out=z_scr[:, bp].unsqueeze(0),
                        in_=z5[:, :, :, bp, :],
                    )

                # gather Z rows: z_scr partition (blk bp g) == natural b
                Z_row = small.tile([BC, N_MEM], BF16, tag="Z_row")
                zq.dma_start(
                    out=Z_row,
                    in_=z_scr.rearrange("blk bp g m -> (blk bp g) m"),
                )

                # softmax over m, batched over all 128 b
                nmx = small.tile([BC, 1], F32, tag="nmx")
                nc.vector.tensor_reduce(
                    out=nmx, in_=Z_row, axis=AX.X, op=ALU.max, negate=True
                )
                e_row = small.tile([BC, N_MEM], F32, tag="e_row")
                nc.scalar.activation(e_row, Z_row, AF.Exp, bias=nmx)
                ssum = small.tile([BC, 1], F32, tag="ssum")
                nc.vector.tensor_reduce(out=ssum, in_=e_row, axis=AX.X, op=ALU.add)
                rsum = small.tile([BC, 1], F32, tag="rsum")
                nc.vector.reciprocal(rsum, ssum)
                g_bf = small.tile([BC, N_MEM], BF16, tag="g_bf")
                nc.vector.tensor_scalar_mul(g_bf, e_row, rsum)

                # o[b,d] = sum_m t[b,d,m] * g[b,m]  (in-place mul into t_hop)
                t3 = t_hop.rearrange("b (d m) -> b d m", d=DIM)
                g3 = g_bf.unsqueeze(1).broadcast_to((BC, DIM, N_MEM))
                nc.vector.tensor_mul(t3, t3, g3)
                for w in (32, 16, 8, 4, 2):
                    nc.vector.tensor_add(
                        t3[:, :, 0:w], t3[:, :, 0:w], t3[:, :, w : 2 * w]
                    )
                o_row = small.tile([BC, DIM], F32, tag="o_row")
                nc.vector.tensor_add(o_row, t3[:, :, 0], t3[:, :, 1])

                # GRU (transposed layout [*, b], fp32)
                ot_ps = pp_g.tile([DIM, BC], F32, tag="gpsum")
                nc.tensor.transpose(ot_ps, o_row, ident)
                oT = small.tile([DIM, BC], F32, tag="oT")
                nc.scalar.copy(out=oT, in_=ot_ps)

                def gate_pair(g):
                    gi = pp_g.tile([DIM, BC], F32, tag="gpsum")
                    nc.tensor.matmul(
                        gi,
                        lhsT=WihT[hop][:, g * DIM : (g + 1) * DIM],
                        rhs=oT,
                        start=True,
                        stop=True,
                    )
                    gh = pp_g.tile([DIM, BC], F32, tag="gpsum")
                    nc.tensor.matmul(
                        gh,
                        lhsT=WhhT[hop][:, g * DIM : (g + 1) * DIM],
                        rhs=MT,
                        start=True,
                        stop=True,
                    )
                    return gi, gh

                rz_t = []
                for g in range(2):
                    gi, gh = gate_pair(g)
                    gb = small.tile([DIM, BC], F32, tag=f"g{g}b")
                    nc.vector.tensor_scalar(
                        out=gb, in0=gi, scalar1=bsum_rz[hop][g], scalar2=None,
                        op0=ALU.add,
                    )
                    nc.vector.tensor_add(gb, gb, gh)
                    gt = small.tile([DIM, BC], F32, tag=f"gate{g}")
                    # sigmoid via tanh keeps every Act func in the
                    # exp_and_others table: no LoadActFuncSet swaps
                    nc.scalar.activation(gt, gb, AF.Tanh, scale=0.5)
                    nc.vector.tensor_scalar(
                        out=gt, in0=gt, scalar1=0.5, scalar2=0.5,
                        op0=ALU.mult, op1=ALU.add,
                    )
                    rz_t.append(gt)
                r_t, z_t = rz_t

                gi_n, gh_n = gate_pair(2)
                ghn = small.tile([DIM, BC], F32, tag="ghn")
                nc.vector.tensor_scalar(
                    out=ghn, in0=gh_n, scalar1=bhhn_t[hop], scalar2=None,
                    op0=ALU.add,
                )
                gin = small.tile([DIM, BC], F32, tag="gin")
                nc.vector.tensor_scalar(
                    out=gin, in0=gi_n, scalar1=bihn_t[hop], scalar2=None,
                    op0=ALU.add,
                )
                n1 = small.tile([DIM, BC], F32, tag="n1")
                nc.vector.tensor_mul(n1, r_t, ghn)
                nc.vector.tensor_add(n1, n1, gin)
                n_t = small.tile([DIM, BC], F32, tag="n_t")
                nc.scalar.activation(n_t, n1, AF.Tanh)

                # M' = n + z * (M - n)
                MT_new = mstate.tile([DIM, BC], F32, tag="MT")
                nc.vector.tensor_sub(MT_new, MT, n_t)
                nc.vector.tensor_mul(MT_new, MT_new, z_t)
                nc.vector.tensor_add(MT_new, MT_new, n_t)
                MT = MT_new

                # M_row for output / M_rep rebuild
                mrow_ps = pp_g.tile([BC, DIM], F32, tag="gpsum")
                nc.tensor.transpose(mrow_ps, MT, ident[:DIM, :DIM])
                M_row = mstate.tile([BC, DIM], F32, tag="M_row")
                nc.scalar.copy(out=M_row, in_=mrow_ps)

                if hop < N_HOP - 1:
                    # rebuild M_rep (bf16) via DRAM bounce
                    Mb = mstate.tile([BC, DIM], BF16, tag="Mb")
                    nc.scalar.copy(out=Mb, in_=M_row)
                    nc.scalar.dma_start(out=m_scr[:, :], in_=Mb)
                    # DRAM->DRAM permute: natural b -> [bp, blk, g, d]
                    nc.scalar.dma_start(
                        out=m_scr2[:, :, :, :],
                        in_=m_scr.rearrange(
                            "(blk bp g) d -> bp blk g d", bp=2, g=NG
                        ),
                    )
                    # partition_broadcast prepends the broadcast dim: in is
                    # [m(bcast), bp, f]; write rows r = bp*64+m accordingly
                    nc.scalar.dma_start(
                        out=m_scr3.rearrange("(bp m) f -> m bp f", bp=2),
                        in_=m_scr2.rearrange(
                            "bp blk g d -> bp (blk g d)"
                        ).partition_broadcast(N_MEM),
                    )
                    M_rep_new = mstate.tile(
                        [128, NBLK * NG * DIM], BF16, tag="M_rep", bufs=1
                    )
                    nc.scalar.dma_start(out=M_rep_new, in_=m_scr3[:, :])
                    M_rep = M_rep_new
                else:
                    nc.sync.dma_start(out=out_d[:, :], in_=M_row)

    nc.compile()
    return nc


_NC_CACHE = None


def _get_nc():
    global _NC_CACHE
    if _NC_CACHE is None:
        _NC_CACHE = build_nc()
    return _NC_CACHE


BF_NP = ml_dtypes.bfloat16


def make_in_maps(hs, Rs, ts, vs, W1, b1, W2, W_ih, W_hh, b_ih, b_hh):
    hs = np.asarray(hs)
    Rs = np.asarray(Rs)
    ts = np.asarray(ts)
    vs = np.asarray(vs)
    W1 = np.asarray(W1)
    b1 = np.asarray(b1)
    W2 = np.asarray(W2)
    W_ih = np.asarray(W_ih)
    W_hh = np.asarray(W_hh)
    b_ih = np.asarray(b_ih)
    b_hh = np.asarray(b_hh)

    # packed bf16 consts [128, 33]: W1T | W2T column
    pk_bf = np.zeros((128, 33), dtype=np.float32)
    pk_bf[:, 0:32] = W1.T
    pk_bf[0:DIM, 32] = W2[0, :]
    pk_bf = pk_bf.astype(BF_NP)

    # packed fp32 consts [128, PF_COLS] (vs slot filled per core below)
    pk_f = np.zeros((128, PF_COLS), dtype=np.float32)
    pk_f[0:DIM, PF_B1] = b1
    for h in range(N_HOP):
        pk_f[0:DIM, PF_W + 192 * h : PF_W + 192 * h + 96] = W_ih[h].T
        pk_f[0:DIM, PF_W + 192 * h + 96 : PF_W + 192 * h + 192] = W_hh[h].T
        for j in range(3):
            pk_f[0:DIM, PF_BIAS + 6 * h + j] = b_ih[h, DIM * j : DIM * (j + 1)]
            pk_f[0:DIM, PF_BIAS + 6 * h + 3 + j] = b_hh[h, DIM * j : DIM * (j + 1)]

    in_maps = []
    for c in range(N_CORES):
        sl = slice(c * BC, (c + 1) * BC)
        # Rs [BC, hop, m, d, e] -> [hop, blk, bp, m, g, d, e]
        Rp = (
            Rs[sl]
            .reshape(NBLK, 2, NG, N_HOP, N_MEM, DIM, DIM)
            .transpose(3, 0, 1, 4, 2, 5, 6)
            .astype(BF_NP)
        )
        # hs [BC, hop, m, e] -> [hop, bp, m, blk, g, e]
        hp = (
            hs[sl]
            .reshape(NBLK, 2, NG, N_HOP, N_MEM, DIM)
            .transpose(3, 1, 4, 0, 2, 5)
            .astype(BF_NP)
        )
        # ts [BC, hop, m, d] -> [BC, hop, d, m]
        tp = ts[sl].transpose(0, 1, 3, 2).astype(BF_NP)
        vsc = np.ascontiguousarray(vs[sl])
        # v_rep [128=(bp,m), (blk, g, d)]
        vv = vsc.reshape(NBLK, 2, NG, DIM).transpose(1, 0, 2, 3)  # [bp, blk, g, d]
        v_rep = (
            np.broadcast_to(vv[:, None], (2, N_MEM, NBLK, NG, DIM))
            .reshape(128, NBLK * NG * DIM)
            .astype(BF_NP)
        )
        pk_fc = pk_f.copy()
        pk_fc[:, PF_VS : PF_VS + 32] = vsc
        in_maps.append(
            {
                "Rs": np.ascontiguousarray(Rp),
                "hs": np.ascontiguousarray(hp),
                "ts": np.ascontiguousarray(tp),
                "v_rep": np.ascontiguousarray(v_rep),
                "pk_bf": pk_bf,
                "pk_f32": pk_fc,
            }
        )
    return in_maps


def kernel(hs, Rs, ts, vs, W1, b1, W2, b2, W_ih, W_hh, b_ih, b_hh):
    from concourse.bass_utils import run_bass_kernel_spmd

    nc = _get_nc()
    in_maps = make_in_maps(hs, Rs, ts, vs, W1, b1, W2, W_ih, W_hh, b_ih, b_hh)
    res = run_bass_kernel_spmd(nc, in_maps, list(range(N_CORES)))
    return np.concatenate([r["out"] for r in res.results], axis=0)
